# revision 35
# baseline (speedup 1.0000x reference)
"""Trainium2 Bass kernel for nn_DANNet2 (moe_routing).

Two-phase routed design over 8 NeuronCores:
  Phase A (data-parallel over batch): bottleneck (BN folded) -> classifier
    log_softmax `src`, plus domain classifier `dclf`. 256 rows/core.
  Host: argmax(src) -> routing labels; gather rows per expert.
  Phase B (expert-parallel): core e runs expert e's 1280->1024->1024->512->2
    LayerNorm MLP on its routed rows (padded to capacity C), log_softmax -> eo.
  Host: scatter eo rows back to sout.

Layout: feature-major activations [128(part)=feature chunk, k, batch] so layer
biases / LN affine are per-partition (free via scalar.activation bias/scale).
LN stats via ones-matmul on PE producing broadcast [128, Bc] mean/E[x^2].
"""
import sys

sys.path.insert(0, "/opt/trn_rl_repo")

import numpy as np

import concourse.bass as bass  # noqa: F401
import concourse.mybir as mybir
import concourse.tile as tile
from concourse import bacc
from concourse.bass_utils import run_bass_kernel_spmd

P = 128
NCORES = 8
B = 2048
D = 1280
E = 8
BSH = B // NCORES  # 256 rows per core in phase A
EPS = 1e-5
F32 = mybir.dt.float32
F32R = mybir.dt.float32r
AF = mybir.ActivationFunctionType


def _chunks(C):
    """Split C columns into matmul chunks <=512, preferring >=256 (float32r
    runs at full rate only when the moving dim is >=256)."""
    sizes = []
    rem = C
    while rem > 512:
        sizes.append(512)
        rem -= 512
    sizes.append(rem)
    if len(sizes) >= 2 and sizes[-1] < 256:
        tail = sizes.pop() + sizes.pop()
        h = ((tail // 2 + 63) // 64) * 64
        sizes += [h, tail - h]
    out, c0 = [], 0
    for bc in sizes:
        out.append((c0, bc))
        c0 += bc
    return out


# ---------------------------------------------------------------- builders

def _cast(ap, dtyp):
    return ap if ap.dtype == dtyp else ap.bitcast(dtyp)


def _mm(nc, ps, lhsT, rhs, start, stop, fast):
    dtyp = F32R if fast else F32
    nc.tensor.matmul(ps, lhsT=_cast(lhsT, dtyp), rhs=_cast(rhs, dtyp),
                     start=start, stop=stop)


def _lin_relu(nc, pool, psum, x_fm, w_s, b_s, kin, mout, NB, tag, fast=False,
              out_round=False):
    """y = relu(W.T x + b); w_s resident [128, kin, mout*128]."""
    y = pool.tile([P, mout, NB], F32, name=f"y_{tag}", tag=f"y_{tag}")
    for m in range(mout):
        for c0, bc in _chunks(NB):
            ps = psum.tile([P, 512], F32, name=f"ps_{tag}_{m}_{c0}", tag="mm", bufs=3)
            for k in range(kin):
                _mm(nc, ps[:, :bc], w_s[:, k, m * P:(m + 1) * P],
                    x_fm[:, k, c0:c0 + bc], k == 0, k == kin - 1, fast)
            yout = y[:, m, c0:c0 + bc]
            if out_round:
                yout = yout.bitcast(F32R)
            nc.scalar.activation(yout, ps[:, :bc], AF.Relu,
                                 bias=b_s[:, m:m + 1], scale=1.0)
    return y


def _ln_mlp_layer(nc, pool, psum, x_fm, w_d, b_s, g_s, t_s, kin, mout, NB,
                  inv_s, eps_s, tag, wgroups=1, out_round=True):
    """y = relu(LN(W.T x + b) * g + t), feature-major. Weights streamed in
    `wgroups` groups of m-chunks, double-buffered."""
    x2 = pool.tile([P, mout, 2, NB], F32, name=f"x2_{tag}", tag="x2")
    y = pool.tile([P, mout, NB], F32, name=f"yl_{tag}", tag="yl", bufs=2)
    if wgroups == 0:
        w_tiles = [(0, mout, w_d)]
    else:
        per = mout // wgroups
        w_tiles = []
        for h in range(wgroups):
            m0 = h * per
            if h == 0 and wg0 is not None:
                w_tiles.append((m0, per, wg0))
                continue
            w_s = pool.tile([P, kin, per * P], F32R, name=f"w_{tag}_{m0}",
                            tag=f"wg_{tag}", bufs=wgroups)
            nc.sync.dma_start(w_s, w_d[:, :, m0 * P:(m0 + per) * P])
            w_tiles.append((m0, per, w_s))
    for c0, bc in _chunks(NB):
        for m0, per, w_s in w_tiles:
            for mi in range(per):
                m = m0 + mi
                col = (mi if wgroups else m)
                ps = psum.tile([P, 512], F32, name=f"ps_{tag}_{m}_{c0}",
                               tag="mm", bufs=3)
                for k in range(kin):
                    _mm(nc, ps[:, :bc], w_s[:, k, col * P:(col + 1) * P],
                        x_fm[:, k, c0:c0 + bc], k == 0, k == kin - 1, True)
                nc.vector.tensor_scalar_add(
                    x2[:, m, 0, c0:c0 + bc].bitcast(F32R), ps[:, :bc],
                    b_s[:, m:m + 1])
                nc.scalar.activation(x2[:, m, 1, c0:c0 + bc].bitcast(F32R),
                                     ps[:, :bc],
                                     AF.Square, bias=b_s[:, m:m + 1],
                                     scale=1.0)
        ps_mu = psum.tile([P, 512], F32, name=f"psmu_{tag}_{c0}", tag="mu")
        ps_sq = psum.tile([P, 512], F32, name=f"pssq_{tag}_{c0}", tag="sq")
        for k in range(mout):
            _mm(nc, ps_mu[:, :bc], inv_s, x2[:, k, 0, c0:c0 + bc],
                k == 0, k == mout - 1, True)
        for k in range(mout):
            _mm(nc, ps_sq[:, :bc], inv_s, x2[:, k, 1, c0:c0 + bc],
                k == 0, k == mout - 1, True)
        mu_s = pool.tile([P, 512], F32, name=f"mu_{tag}_{c0}", tag="stat",
                         bufs=4)
        nc.vector.tensor_copy(mu_s[:, :bc], ps_mu[:, :bc])
        tmp_s = pool.tile([P, 512], F32, name=f"tmp_{tag}_{c0}", tag="stat",
                          bufs=4)
        nc.vector.tensor_mul(tmp_s[:, :bc], mu_s[:, :bc], mu_s[:, :bc])
        nc.vector.tensor_sub(tmp_s[:, :bc], ps_sq[:, :bc], tmp_s[:, :bc])
        rstd_s = pool.tile([P, 512], F32, name=f"rstd_{tag}_{c0}", tag="stat",
                           bufs=4)
        nc.scalar.activation(rstd_s[:, :bc], tmp_s[:, :bc],
                             AF.Abs_reciprocal_sqrt, bias=eps_s, scale=1.0)
        for m in range(mout):
            t1 = pool.tile([P, 512], F32, name=f"nrm_{tag}_{m}_{c0}",
                           tag="nrm", bufs=2)
            nc.vector.tensor_sub(t1[:, :bc], x2[:, m, 0, c0:c0 + bc],
                                 mu_s[:, :bc])
            nc.vector.tensor_mul(t1[:, :bc], t1[:, :bc], rstd_s[:, :bc])
            yout = y[:, m, c0:c0 + bc]
            if out_round:
                yout = yout.bitcast(F32R)
            nc.scalar.activation(yout, t1[:, :bc], AF.Relu,
                                 bias=t_s[:, m:m + 1], scale=g_s[:, m:m + 1])
    return y


def _head_logsoftmax(nc, pool, psum, x_fm, w_s_chunks, b_s, out_d, kin, nout,
                     NB, ones_row, tag, fast=False):
    """out = log_softmax(x.T W + b) row-wise, batch-major out [NB, nout]."""
    assert NB % P == 0
    for bt in range(NB // P):
        ps = psum.tile([P, nout], F32, name=f"hps_{tag}_{bt}", tag="mm", bufs=3)
        nc.tensor.matmul(ps, lhsT=ones_row, rhs=b_s, start=True, stop=False)
        for k in range(kin):
            _mm(nc, ps, x_fm[:, k, bt * P:(bt + 1) * P], w_s_chunks[k],
                False, k == kin - 1, fast)
        mx = pool.tile([P, 1], F32, name=f"mx_{tag}_{bt}", tag="h1", bufs=4)
        nc.vector.tensor_reduce(mx, ps, axis=mybir.AxisListType.X,
                                op=mybir.AluOpType.max)
        nmx = pool.tile([P, 1], F32, name=f"nmx_{tag}_{bt}", tag="h2", bufs=4)
        nc.vector.tensor_scalar_mul(nmx, mx, -1.0)
        ex = pool.tile([P, nout], F32, name=f"ex_{tag}_{bt}", tag="h3", bufs=4)
        se = pool.tile([P, 1], F32, name=f"se_{tag}_{bt}", tag="h4", bufs=4)
        nc.scalar.activation(ex, ps, AF.Exp, bias=nmx, scale=1.0,
                             accum_out=se)
        ls = pool.tile([P, 1], F32, name=f"ls_{tag}_{bt}", tag="h5", bufs=4)
        nc.scalar.activation(ls, se, AF.Ln, bias=0.0, scale=1.0)
        sh = pool.tile([P, 1], F32, name=f"sh_{tag}_{bt}", tag="h6", bufs=4)
        nc.vector.tensor_add(sh, mx, ls)
        nc.vector.tensor_scalar_mul(sh, sh, -1.0)
        ob = pool.tile([P, nout], F32, name=f"ob_{tag}_{bt}", tag="h7", bufs=4)
        nc.scalar.activation(ob, ps, AF.Identity, bias=sh, scale=1.0)
        nc.sync.dma_start(out_d[bt * P:(bt + 1) * P, :], ob)


def _head2_logsoftmax(nc, pool, psum, x_fm, wdiff_s, bdiff_s, out_d, kin, NB,
                      ones_row, tag):
    """2-class log_softmax via softplus of the logit difference d=a-b:
    out = [-softplus(-d), -softplus(d)]. Exact same math, short ladder."""
    assert NB % P == 0
    for bt in range(NB // P):
        ps = psum.tile([P, 1], F32, name=f"h2ps_{tag}_{bt}", tag="sq", bufs=2)
        nc.tensor.matmul(ps, lhsT=ones_row, rhs=bdiff_s, start=True,
                         stop=False)
        for k in range(kin):
            nc.tensor.matmul(ps, lhsT=x_fm[:, k, bt * P:(bt + 1) * P],
                             rhs=wdiff_s[k], start=False, stop=(k == kin - 1))
        # la = log(1 + exp(-d));  out = [-la, -(la + d)]
        ex = pool.tile([P, 1], F32, name=f"ex_{tag}_{bt}", tag="h2a", bufs=4)
        nc.scalar.activation(ex, ps, AF.Exp, bias=0.0, scale=-1.0)
        la = pool.tile([P, 1], F32, name=f"la_{tag}_{bt}", tag="h2b", bufs=4)
        nc.scalar.activation(la, ex, AF.Ln, bias=one_s, scale=1.0)
        ob = pool.tile([P, 2], F32, name=f"o2_{tag}_{bt}", tag="h2o", bufs=4)
        nc.vector.tensor_scalar_mul(ob[:, 0:1], la, -1.0)
        lb = pool.tile([P, 1], F32, name=f"lb_{tag}_{bt}", tag="h2c", bufs=4)
        nc.vector.tensor_add(lb, la, ps)
        nc.vector.tensor_scalar_mul(ob[:, 1:2], lb, -1.0)
        nc.sync.dma_start(out_d[bt * P:(bt + 1) * P, :], ob)


def _build_phase_a():
    nc = bacc.Bacc("TRN2", target_bir_lowering=False, debug=False,
                   num_devices=NCORES)
    dt = lambda n, s, k="ExternalInput", ty=F32: \
        nc.dram_tensor(n, s, ty, kind=k).ap()
    xt = dt("xt", [P, D // P, BSH], ty=F32R)
    w1 = dt("w1", [P, D // P, 512], ty=F32R);   b1 = dt("b1", [P, 4])
    w2 = dt("w2", [P, 4, 256], ty=F32R);        b2 = dt("b2", [P, 2])
    cw = dt("cw", [2, P, E], ty=F32R);          cb = dt("cb", [1, E])
    dw1 = dt("dw1", [P, D // P, 1024], ty=F32R); db1 = dt("db1", [P, 8])
    dw2 = dt("dw2", [P, 8, 1024], ty=F32R);      db2 = dt("db2", [P, 8])
    dw3 = dt("dw3", [P, 8, 512], ty=F32R);       db3 = dt("db3", [P, 4])
    dw4 = dt("dw4", [4, P, 2]);         db4 = dt("db4", [1, 2])
    dwd = dt("dwd", [4, P, 1]);         dbd = dt("dbd", [1, 1])
    src_d = dt("src", [BSH, E], "ExternalOutput")
    dclf_d = dt("dclf", [BSH, 2], "ExternalOutput")

    with tile.TileContext(nc) as tc:
        with tc.tile_pool(name="pool", bufs=1) as pool, \
             tc.tile_pool(name="psum", bufs=2, space="PSUM") as psum, \
             tc.tile_pool(name="const", bufs=1) as const:
            ones_row = const.tile([1, P], F32)
            nc.vector.memset(ones_row, 1.0)
            xt_s = pool.tile([P, D // P, BSH], F32, name="xt_s", tag="xt")
            nc.sync.dma_start(xt_s, xt)
            # resident weights, loaded up front in big DMAs
            w1_s = pool.tile([P, D // P, 512], F32, name="w1_s", tag="w1")
            nc.sync.dma_start(w1_s, w1)
            w2_s = pool.tile([P, 4, 256], F32, name="w2_s", tag="w2")
            nc.sync.dma_start(w2_s, w2)
            dw1_s = pool.tile([P, D // P, 1024], F32, name="dw1_s", tag="dw1")
            nc.sync.dma_start(dw1_s, dw1)
            dw2_s = pool.tile([P, 8, 1024], F32, name="dw2_s", tag="dw2")
            nc.sync.dma_start(dw2_s, dw2)
            dw3_s = pool.tile([P, 8, 512], F32, name="dw3_s", tag="dw3")
            nc.sync.dma_start(dw3_s, dw3)
            small = {}
            for nm, d_ap, shp in [("b1", b1, [P, 4]), ("b2", b2, [P, 2]),
                                  ("db1", db1, [P, 8]), ("db2", db2, [P, 8]),
                                  ("db3", db3, [P, 4]), ("cb", cb, [1, E]),
                                  ("db4", db4, [1, 2])]:
                t = pool.tile(shp, F32, name=nm + "s", tag=nm + "s")
                nc.sync.dma_start(t, d_ap)
                small[nm] = t
            cw_s = pool.tile([P, 2, E], F32, name="cws", tag="cws")
            nc.sync.dma_start(cw_s, cw.rearrange("k p e -> p k e"))
            dw4_s = pool.tile([P, 4, 2], F32, name="dw4s", tag="dw4s")
            nc.sync.dma_start(dw4_s, dw4.rearrange("k p e -> p k e"))

            h1 = _lin_relu(nc, pool, psum, xt_s, w1_s, small["b1"], D // P, 4,
                           BSH, "h1")
            h2 = _lin_relu(nc, pool, psum, h1, w2_s, small["b2"], 4, 2,
                           BSH, "h2")
            _head_logsoftmax(nc, pool, psum, h2,
                             [cw_s[:, k, :] for k in range(2)],
                             small["cb"], src_d, 2, E, BSH, ones_row, "src")
            d1 = _lin_relu(nc, pool, psum, xt_s, dw1_s, small["db1"], D // P,
                           8, BSH, "d1", fast=True, out_round=True)
            d2 = _lin_relu(nc, pool, psum, d1, dw2_s, small["db2"], 8, 8,
                           BSH, "d2", fast=True, out_round=True)
            d3 = _lin_relu(nc, pool, psum, d2, dw3_s, small["db3"], 8, 4,
                           BSH, "d3", fast=True)
            _head_logsoftmax(nc, pool, psum, d3,
                             [dw4_s[:, k, :] for k in range(4)],
                             small["db4"], dclf_d, 4, 2, BSH, ones_row, "dclf")
    nc.compile()
    return nc


def _build_phase_b(C):
    nc = bacc.Bacc("TRN2", target_bir_lowering=False, debug=False,
                   num_devices=NCORES)
    dt = lambda n, s, k="ExternalInput", ty=F32: \
        nc.dram_tensor(n, s, ty, kind=k).ap()
    xg = dt("xg", [P, D // P, C], ty=F32R)
    we1 = dt("we1", [P, D // P, 1024], ty=F32R)
    we2 = dt("we2", [P, 8, 1024], ty=F32R)
    we3 = dt("we3", [P, 8, 512], ty=F32R)
    we4 = dt("we4", [4, P, 2])
    wed = dt("wed", [4, P, 1]);  ebd = dt("ebd", [1, 1])
    eb1 = dt("eb1", [P, 8]); eg1 = dt("eg1", [P, 8]); et1 = dt("et1", [P, 8])
    eb2 = dt("eb2", [P, 8]); eg2 = dt("eg2", [P, 8]); et2 = dt("et2", [P, 8])
    eb3 = dt("eb3", [P, 4]); eg3 = dt("eg3", [P, 4]); et3 = dt("et3", [P, 4])
    eb4 = dt("eb4", [1, 2])
    eo_d = dt("eo", [C, 2], "ExternalOutput")

    with tile.TileContext(nc) as tc:
        with tc.tile_pool(name="pool", bufs=1) as pool, \
             tc.tile_pool(name="psum", bufs=2, space="PSUM") as psum, \
             tc.tile_pool(name="const", bufs=1) as const:
            ones_row = const.tile([1, P], F32)
            nc.vector.memset(ones_row, 1.0)
            inv1024 = const.tile([P, P], F32)
            nc.vector.memset(inv1024, 1.0 / 1024.0)
            inv512 = const.tile([P, P], F32)
            nc.vector.memset(inv512, 1.0 / 512.0)
            eps_s = const.tile([P, 1], F32)
            nc.vector.memset(eps_s, EPS)
            xg_s = pool.tile([P, D // P, C], F32R, name="xg_s", tag="xg")
            nc.sync.dma_start(xg_s, xg)
            small = {}
            for nm, d_ap, sh in [("eb1", eb1, 8), ("eg1", eg1, 8),
                                 ("et1", et1, 8), ("eb2", eb2, 8),
                                 ("eg2", eg2, 8), ("et2", et2, 8),
                                 ("eb3", eb3, 4), ("eg3", eg3, 4),
                                 ("et3", et3, 4)]:
                t = pool.tile([P, sh], F32, name=nm + "s", tag=nm + "s")
                nc.sync.dma_start(t, d_ap)
                small[nm] = t
            eb4_s = pool.tile([1, 2], F32, name="eb4s", tag="eb4s")
            nc.sync.dma_start(eb4_s, eb4)
            we4_s = pool.tile([P, 4, 2], F32, name="we4s", tag="we4s")
            nc.sync.dma_start(we4_s, we4.rearrange("k p e -> p k e"))
            # we2 resident, loaded early to overlap with L1 compute
            we2_s = pool.tile([P, 8, 1024], F32R, name="we2_s", tag="we2")
            nc.sync.dma_start(we2_s, we2)

            x1 = _ln_mlp_layer(nc, pool, psum, xg_s, we1, small["eb1"],
                               small["eg1"], small["et1"], D // P, 8, C,
                               inv1024, eps_s, "e1", wgroups=2)
            x2 = _ln_mlp_layer(nc, pool, psum, x1, we2_s, small["eb2"],
                               small["eg2"], small["et2"], 8, 8, C,
                               inv1024, eps_s, "e2", wgroups=0)
            x3 = _ln_mlp_layer(nc, pool, psum, x2, we3, small["eb3"],
                               small["eg3"], small["et3"], 8, 4, C,
                               inv512, eps_s, "e3", wgroups=2, out_round=False)
            _head_logsoftmax(nc, pool, psum, x3,
                             [we4_s[:, k, :] for k in range(4)],
                             eb4_s, eo_d, 4, 2, C, ones_row, "eo")
    nc.compile()
    return nc


_BUILT = {}
_PREP = {}
_LAST_EXEC_NS = None  # sum of per-phase max-core exec times, when traced


def _fingerprint(arrs):
    h = 0
    for a in arrs:
        b = np.ascontiguousarray(a).view(np.uint8)
        h = hash((h, a.shape, bytes(b[:: max(1, b.size // 64)][:64].tobytes())))
    return h


def _run(nc, maps):
    """run_bass_kernel_spmd with graceful fallback when the axon NTFF
    profiling hook is unavailable but BASS_TRACE is set."""
    import os
    try:
        return run_bass_kernel_spmd(nc, maps, core_ids=list(range(NCORES)))
    except ModuleNotFoundError:
        os.environ["BASS_NEVER_TRACE"] = "1"
        return run_bass_kernel_spmd(nc, maps, core_ids=list(range(NCORES)))


def _get(key, fn):
    if key not in _BUILT:
        _BUILT[key] = fn()
    return _BUILT[key]


# ---------------------------------------------------------------- host prep

def _prep_w(w):
    """[K, F] -> [128(=ki part), K//128, F] contiguous."""
    K, F = w.shape
    return np.ascontiguousarray(w.reshape(K // P, P, F).transpose(1, 0, 2))


def _prep_b(b):
    F = b.shape[0]
    return np.ascontiguousarray(b.reshape(F // P, P).T)


def _prep_xt(x):
    """[N, D] -> [128, D//128, N] feature-major."""
    N = x.shape[0]
    return np.ascontiguousarray(
        x.T.reshape(D // P, P, N).transpose(1, 0, 2))


def _fold_bn(w, b, g, beta, m, v):
    a = (g / np.sqrt(v + EPS)).astype(np.float32)
    return (w * a[None, :]).astype(np.float32), \
        ((b - m) * a + beta).astype(np.float32)


def kernel(**inputs):
    f = {k: np.asarray(v, dtype=np.float32) for k, v in inputs.items()}
    source = f["source"]

    # ---- phase A inputs
    w1f, b1f = _fold_bn(f["bw1"], f["bb1"], f["bg1"], f["bbe1"], f["bm1"], f["bv1"])
    w2f, b2f = _fold_bn(f["bw2"], f["bb2"], f["bg2"], f["bbe2"], f["bm2"], f["bv2"])
    d1f, db1f = _fold_bn(f["dw1"], f["db1"], f["dg1"], f["dbe1"], f["dm1"], f["dv1"])
    d2f, db2f = _fold_bn(f["dw2"], f["db2"], f["dg2"], f["dbe2"], f["dm2"], f["dv2"])
    d3f, db3f = _fold_bn(f["dw3"], f["db3"], f["dg3"], f["dbe3"], f["dm3"], f["dv3"])
    shared = {
        "w1": _prep_w(w1f), "b1": _prep_b(b1f),
        "w2": _prep_w(w2f), "b2": _prep_b(b2f),
        "cw": np.ascontiguousarray(f["cw"].reshape(2, P, E)),
        "cb": f["cb"].reshape(1, E).copy(),
        "dw1": _prep_w(d1f), "db1": _prep_b(db1f),
        "dw2": _prep_w(d2f), "db2": _prep_b(db2f),
        "dw3": _prep_w(d3f), "db3": _prep_b(db3f),
        "dw4": np.ascontiguousarray(f["dw4"].reshape(4, P, 2)),
        "db4": f["db4"].reshape(1, 2).copy(),
    }
    nc_a = _get("a", _build_phase_a)
    in_maps_a = []
    for c in range(NCORES):
        m = dict(shared)
        m["xt"] = _prep_xt(source[c * BSH:(c + 1) * BSH])
        in_maps_a.append(m)
    res_a = _run(nc_a, in_maps_a)
    src = np.concatenate([r["src"] for r in res_a.results], axis=0)
    dclf = np.concatenate([r["dclf"] for r in res_a.results], axis=0)

    # ---- routing on host (argmax of device-computed log-softmax)
    label = np.argmax(src, axis=-1)
    idx = [np.nonzero(label == e)[0] for e in range(E)]
    maxn = max(1, max(len(i) for i in idx))
    C = max(256, ((maxn + 63) // 64) * 64)

    nc_b = _get(("b", C), lambda: _build_phase_b(C))
    in_maps_b = []
    for e in range(E):
        g = np.zeros((C, D), dtype=np.float32)
        g[:len(idx[e])] = source[idx[e]]
        in_maps_b.append({
            "xg": _prep_xt(g),
            "we1": _prep_w(f["ew1"][e]), "we2": _prep_w(f["ew2"][e]),
            "we3": _prep_w(f["ew3"][e]),
            "we4": np.ascontiguousarray(f["ew4"][e].reshape(4, P, 2)),
            "eb1": _prep_b(f["eb1"][e]), "eg1": _prep_b(f["eg1"][e]),
            "et1": _prep_b(f["ebe1"][e]),
            "eb2": _prep_b(f["eb2"][e]), "eg2": _prep_b(f["eg2"][e]),
            "et2": _prep_b(f["ebe2"][e]),
            "eb3": _prep_b(f["eb3"][e]), "eg3": _prep_b(f["eg3"][e]),
            "et3": _prep_b(f["ebe3"][e]),
            "eb4": f["eb4"][e].reshape(1, 2).copy(),
        })
    res_b = _run(nc_b, in_maps_b)
    sout = np.zeros((B, 2), dtype=np.float32)
    for e in range(E):
        if len(idx[e]):
            sout[idx[e]] = res_b.results[e]["eo"][:len(idx[e])]

    global _LAST_EXEC_NS
    if res_a.exec_time_ns is not None and res_b.exec_time_ns is not None:
        _LAST_EXEC_NS = res_a.exec_time_ns + res_b.exec_time_ns
    return src, dclf, sout, source


# revision 38
# speedup vs baseline: 1.0828x; 1.0828x over previous
"""Trainium2 Bass kernel for nn_DANNet2 (moe_routing).

Two-phase routed design over 8 NeuronCores:
  Phase A (data-parallel over batch): bottleneck (BN folded) -> classifier
    log_softmax `src`, plus domain classifier `dclf`. 256 rows/core.
  Host: argmax(src) -> routing labels; gather rows per expert.
  Phase B (expert-parallel): core e runs expert e's 1280->1024->1024->512->2
    LayerNorm MLP on its routed rows (padded to capacity C), log_softmax -> eo.
  Host: scatter eo rows back to sout.

Layout: feature-major activations [128(part)=feature chunk, k, batch] so layer
biases / LN affine are per-partition (free via scalar.activation bias/scale).
LN stats via ones-matmul on PE producing broadcast [128, Bc] mean/E[x^2].
"""
import sys

sys.path.insert(0, "/opt/trn_rl_repo")

import ml_dtypes
import numpy as np

import concourse.bass as bass  # noqa: F401
import concourse.mybir as mybir
import concourse.tile as tile
from concourse import bacc
from concourse.bass_utils import run_bass_kernel_spmd

P = 128
NCORES = 8
B = 2048
D = 1280
E = 8
BSH = B // NCORES  # 256 rows per core in phase A
EPS = 1e-5
F32 = mybir.dt.float32
F32R = mybir.dt.float32r
BF16 = mybir.dt.bfloat16
AF = mybir.ActivationFunctionType


def _chunks(C):
    """Split C columns into matmul chunks <=512, preferring >=256 (float32r
    runs at full rate only when the moving dim is >=256)."""
    sizes = []
    rem = C
    while rem > 512:
        sizes.append(512)
        rem -= 512
    sizes.append(rem)
    if len(sizes) >= 2 and sizes[-1] < 256:
        tail = sizes.pop() + sizes.pop()
        h = ((tail // 2 + 63) // 64) * 64
        sizes += [h, tail - h]
    out, c0 = [], 0
    for bc in sizes:
        out.append((c0, bc))
        c0 += bc
    return out


# ---------------------------------------------------------------- builders

def _cast(ap, dtyp):
    return ap if ap.dtype == dtyp else ap.bitcast(dtyp)


def _mm(nc, ps, lhsT, rhs, start, stop, fast):
    dtyp = F32R if fast else F32
    nc.tensor.matmul(ps, lhsT=_cast(lhsT, dtyp), rhs=_cast(rhs, dtyp),
                     start=start, stop=stop)


def _lin_relu(nc, pool, psum, x_fm, w_s, b_s, kin, mout, NB, tag, fast=False,
              out_round=False):
    """y = relu(W.T x + b); w_s resident [128, kin, mout*128]."""
    y = pool.tile([P, mout, NB], F32, name=f"y_{tag}", tag=f"y_{tag}")
    for m in range(mout):
        for c0, bc in _chunks(NB):
            ps = psum.tile([P, 512], F32, name=f"ps_{tag}_{m}_{c0}", tag="mm", bufs=4)
            for k in range(kin):
                _mm(nc, ps[:, :bc], w_s[:, k, m * P:(m + 1) * P],
                    x_fm[:, k, c0:c0 + bc], k == 0, k == kin - 1, fast)
            yout = y[:, m, c0:c0 + bc]
            if out_round:
                yout = yout.bitcast(F32R)
            nc.scalar.activation(yout, ps[:, :bc], AF.Relu,
                                 bias=b_s[:, m:m + 1], scale=1.0)
    return y


def _ln_mlp_layer(nc, pool, psum, x_fm, w_d, b_s, g_s, t_s, kin, mout, NB,
                  inv_s, eps_s, tag, wgroups=1, out_round=True):
    """y = relu(LN(W.T x + b) * g + t), feature-major. Weights streamed in
    `wgroups` groups of m-chunks, double-buffered."""
    x2 = pool.tile([P, mout, 2, NB], F32, name=f"x2_{tag}", tag="x2")
    y = pool.tile([P, mout, NB], F32, name=f"yl_{tag}", tag="yl", bufs=2)
    if wgroups == 0:
        w_tiles = [(0, mout, w_d)]
    else:
        per = mout // wgroups
        w_tiles = []
        for h in range(wgroups):
            m0 = h * per
            if h == 0 and wg0 is not None:
                w_tiles.append((m0, per, wg0))
                continue
            w_s = pool.tile([P, kin, per * P], F32R, name=f"w_{tag}_{m0}",
                            tag=f"wg_{tag}", bufs=wgroups)
            nc.sync.dma_start(w_s, w_d[:, :, m0 * P:(m0 + per) * P])
            w_tiles.append((m0, per, w_s))
    for c0, bc in _chunks(NB):
        for m0, per, w_s in w_tiles:
            for mi in range(per):
                m = m0 + mi
                col = (mi if wgroups else m)
                ps = psum.tile([P, 512], F32, name=f"ps_{tag}_{m}_{c0}",
                               tag="mm", bufs=4)
                for k in range(kin):
                    _mm(nc, ps[:, :bc], w_s[:, k, col * P:(col + 1) * P],
                        x_fm[:, k, c0:c0 + bc], k == 0, k == kin - 1, True)
                nc.scalar.activation(x2[:, m, 0, c0:c0 + bc].bitcast(F32R),
                                     ps[:, :bc], AF.Identity,
                                     bias=b_s[:, m:m + 1], scale=1.0)
                nc.scalar.activation(x2[:, m, 1, c0:c0 + bc].bitcast(F32R),
                                     ps[:, :bc],
                                     AF.Square, bias=b_s[:, m:m + 1],
                                     scale=1.0)
        ps_mu = psum.tile([P, 512], F32, name=f"psmu_{tag}_{c0}", tag="mu")
        ps_sq = psum.tile([P, 512], F32, name=f"pssq_{tag}_{c0}", tag="sq")
        for k in range(mout):
            _mm(nc, ps_mu[:, :bc], inv_s, x2[:, k, 0, c0:c0 + bc],
                k == 0, k == mout - 1, True)
        for k in range(mout):
            _mm(nc, ps_sq[:, :bc], inv_s, x2[:, k, 1, c0:c0 + bc],
                k == 0, k == mout - 1, True)
        mu_s = pool.tile([P, 512], F32, name=f"mu_{tag}_{c0}", tag="stat",
                         bufs=4)
        nc.vector.tensor_copy(mu_s[:, :bc], ps_mu[:, :bc])
        tmp_s = pool.tile([P, 512], F32, name=f"tmp_{tag}_{c0}", tag="stat",
                          bufs=4)
        nc.vector.tensor_mul(tmp_s[:, :bc], mu_s[:, :bc], mu_s[:, :bc])
        nc.vector.tensor_sub(tmp_s[:, :bc], ps_sq[:, :bc], tmp_s[:, :bc])
        rstd_s = pool.tile([P, 512], F32, name=f"rstd_{tag}_{c0}", tag="stat",
                           bufs=4)
        nc.scalar.activation(rstd_s[:, :bc], tmp_s[:, :bc],
                             AF.Abs_reciprocal_sqrt, bias=eps_s, scale=1.0)
        for m in range(mout):
            t1 = pool.tile([P, 512], F32, name=f"nrm_{tag}_{m}_{c0}",
                           tag="nrm", bufs=2)
            nc.vector.tensor_sub(t1[:, :bc], x2[:, m, 0, c0:c0 + bc],
                                 mu_s[:, :bc])
            nc.vector.tensor_mul(t1[:, :bc], t1[:, :bc], rstd_s[:, :bc])
            yout = y[:, m, c0:c0 + bc]
            if out_round:
                yout = yout.bitcast(F32R)
            nc.scalar.activation(yout, t1[:, :bc], AF.Relu,
                                 bias=t_s[:, m:m + 1], scale=g_s[:, m:m + 1])
    return y


def _head_logsoftmax(nc, pool, psum, x_fm, w_s_chunks, b_s, out_d, kin, nout,
                     NB, ones_row, tag, fast=False):
    """out = log_softmax(x.T W + b) row-wise, batch-major out [NB, nout]."""
    assert NB % P == 0
    for bt in range(NB // P):
        ps = psum.tile([P, nout], F32, name=f"hps_{tag}_{bt}", tag="mm", bufs=4)
        nc.tensor.matmul(ps, lhsT=ones_row, rhs=b_s, start=True, stop=False)
        for k in range(kin):
            _mm(nc, ps, x_fm[:, k, bt * P:(bt + 1) * P], w_s_chunks[k],
                False, k == kin - 1, fast)
        mx = pool.tile([P, 1], F32, name=f"mx_{tag}_{bt}", tag="h1", bufs=4)
        nc.vector.tensor_reduce(mx, ps, axis=mybir.AxisListType.X,
                                op=mybir.AluOpType.max)
        nmx = pool.tile([P, 1], F32, name=f"nmx_{tag}_{bt}", tag="h2", bufs=4)
        nc.vector.tensor_scalar_mul(nmx, mx, -1.0)
        ex = pool.tile([P, nout], F32, name=f"ex_{tag}_{bt}", tag="h3", bufs=4)
        se = pool.tile([P, 1], F32, name=f"se_{tag}_{bt}", tag="h4", bufs=4)
        nc.scalar.activation(ex, ps, AF.Exp, bias=nmx, scale=1.0,
                             accum_out=se)
        ls = pool.tile([P, 1], F32, name=f"ls_{tag}_{bt}", tag="h5", bufs=4)
        nc.scalar.activation(ls, se, AF.Ln, bias=0.0, scale=1.0)
        sh = pool.tile([P, 1], F32, name=f"sh_{tag}_{bt}", tag="h6", bufs=4)
        nc.vector.tensor_add(sh, mx, ls)
        nc.vector.tensor_scalar_mul(sh, sh, -1.0)
        ob = pool.tile([P, nout], F32, name=f"ob_{tag}_{bt}", tag="h7", bufs=4)
        nc.scalar.activation(ob, ps, AF.Identity, bias=sh, scale=1.0)
        nc.sync.dma_start(out_d[bt * P:(bt + 1) * P, :], ob)


def _head2_logsoftmax(nc, pool, psum, x_fm, wdiff_s, bdiff_s, out_d, kin, NB,
                      ones_row, tag):
    """2-class log_softmax via softplus of the logit difference d=a-b:
    out = [-softplus(-d), -softplus(d)]. Exact same math, short ladder."""
    assert NB % P == 0
    for bt in range(NB // P):
        ps = psum.tile([P, 1], F32, name=f"h2ps_{tag}_{bt}", tag="sq", bufs=2)
        nc.tensor.matmul(ps, lhsT=ones_row, rhs=bdiff_s, start=True,
                         stop=False)
        for k in range(kin):
            nc.tensor.matmul(ps, lhsT=x_fm[:, k, bt * P:(bt + 1) * P],
                             rhs=wdiff_s[k], start=False, stop=(k == kin - 1))
        # la = log(1 + exp(-d));  out = [-la, -(la + d)]
        ex = pool.tile([P, 1], F32, name=f"ex_{tag}_{bt}", tag="h2a", bufs=4)
        nc.scalar.activation(ex, ps, AF.Exp, bias=0.0, scale=-1.0)
        la = pool.tile([P, 1], F32, name=f"la_{tag}_{bt}", tag="h2b", bufs=4)
        nc.scalar.activation(la, ex, AF.Ln, bias=one_s, scale=1.0)
        ob = pool.tile([P, 2], F32, name=f"o2_{tag}_{bt}", tag="h2o", bufs=4)
        nc.vector.tensor_scalar_mul(ob[:, 0:1], la, -1.0)
        lb = pool.tile([P, 1], F32, name=f"lb_{tag}_{bt}", tag="h2c", bufs=4)
        nc.vector.tensor_add(lb, la, ps)
        nc.vector.tensor_scalar_mul(ob[:, 1:2], lb, -1.0)
        nc.sync.dma_start(out_d[bt * P:(bt + 1) * P, :], ob)


def _build_phase_a():
    nc = bacc.Bacc("TRN2", target_bir_lowering=False, debug=False,
                   num_devices=NCORES)
    dt = lambda n, s, k="ExternalInput", ty=F32: \
        nc.dram_tensor(n, s, ty, kind=k).ap()
    xt = dt("xt", [P, D // P, BSH], ty=F32R)
    w1 = dt("w1", [P, D // P, 512], ty=F32R);   b1 = dt("b1", [P, 4])
    w2 = dt("w2", [P, 4, 256], ty=F32R);        b2 = dt("b2", [P, 2])
    cw = dt("cw", [2, P, E], ty=F32R);          cb = dt("cb", [1, E])
    dw1 = dt("dw1", [P, D // P, 1024], ty=BF16); db1 = dt("db1", [P, 8])
    dw2 = dt("dw2", [P, 8, 1024], ty=BF16);      db2 = dt("db2", [P, 8])
    dw3 = dt("dw3", [P, 8, 512], ty=BF16);       db3 = dt("db3", [P, 4])
    dw4 = dt("dw4", [4, P, 2]);         db4 = dt("db4", [1, 2])
    dwd = dt("dwd", [4, P, 1]);         dbd = dt("dbd", [1, 1])
    src_d = dt("src", [BSH, E], "ExternalOutput")
    dclf_d = dt("dclf", [BSH, 2], "ExternalOutput")

    with tile.TileContext(nc) as tc:
        with tc.tile_pool(name="pool", bufs=1) as pool, \
             tc.tile_pool(name="psum", bufs=2, space="PSUM") as psum, \
             tc.tile_pool(name="const", bufs=1) as const:
            ones_row = const.tile([1, P], F32)
            nc.vector.memset(ones_row, 1.0)
            xt_s = pool.tile([P, D // P, BSH], F32, name="xt_s", tag="xt")
            nc.sync.dma_start(xt_s, xt)
            # resident weights, loaded up front in big DMAs
            w1_s = pool.tile([P, D // P, 512], F32, name="w1_s", tag="w1")
            nc.sync.dma_start(w1_s, w1)
            w2_s = pool.tile([P, 4, 256], F32, name="w2_s", tag="w2")
            nc.sync.dma_start(w2_s, w2)
            dw1_s = pool.tile([P, D // P, 1024], F32, name="dw1_s", tag="dw1")
            nc.sync.dma_start(dw1_s, dw1)
            dw2_s = pool.tile([P, 8, 1024], F32, name="dw2_s", tag="dw2")
            nc.sync.dma_start(dw2_s, dw2)
            dw3_s = pool.tile([P, 8, 512], F32, name="dw3_s", tag="dw3")
            nc.sync.dma_start(dw3_s, dw3)
            small = {}
            for nm, d_ap, shp in [("b1", b1, [P, 4]), ("b2", b2, [P, 2]),
                                  ("db1", db1, [P, 8]), ("db2", db2, [P, 8]),
                                  ("db3", db3, [P, 4]), ("cb", cb, [1, E]),
                                  ("db4", db4, [1, 2])]:
                t = pool.tile(shp, F32, name=nm + "s", tag=nm + "s")
                nc.sync.dma_start(t, d_ap)
                small[nm] = t
            cw_s = pool.tile([P, 2, E], F32, name="cws", tag="cws")
            nc.sync.dma_start(cw_s, cw.rearrange("k p e -> p k e"))
            dw4_s = pool.tile([P, 4, 2], F32, name="dw4s", tag="dw4s")
            nc.sync.dma_start(dw4_s, dw4.rearrange("k p e -> p k e"))

            h1 = _lin_relu(nc, pool, psum, xt_s, w1_s, small["b1"], D // P, 4,
                           BSH, "h1")
            h2 = _lin_relu(nc, pool, psum, h1, w2_s, small["b2"], 4, 2,
                           BSH, "h2")
            _head_logsoftmax(nc, pool, psum, h2,
                             [cw_s[:, k, :] for k in range(2)],
                             small["cb"], src_d, 2, E, BSH, ones_row, "src")
            d1 = _lin_relu(nc, pool, psum, xt_s, dw1_s, small["db1"], D // P,
                           8, BSH, "d1", fast=True, out_round=True)
            d2 = _lin_relu(nc, pool, psum, d1, dw2_s, small["db2"], 8, 8,
                           BSH, "d2", fast=True, out_round=True)
            d3 = _lin_relu(nc, pool, psum, d2, dw3_s, small["db3"], 8, 4,
                           BSH, "d3", fast=True)
            _head_logsoftmax(nc, pool, psum, d3,
                             [dw4_s[:, k, :] for k in range(4)],
                             small["db4"], dclf_d, 4, 2, BSH, ones_row, "dclf")
    nc.compile()
    return nc


def _build_phase_b(C):
    nc = bacc.Bacc("TRN2", target_bir_lowering=False, debug=False,
                   num_devices=NCORES)
    dt = lambda n, s, k="ExternalInput", ty=F32: \
        nc.dram_tensor(n, s, ty, kind=k).ap()
    xg = dt("xg", [P, D // P, C], ty=F32R)
    we1 = dt("we1", [P, D // P, 1024], ty=F32R)
    we2 = dt("we2", [P, 8, 1024], ty=F32R)
    we3 = dt("we3", [P, 8, 512], ty=F32R)
    we4 = dt("we4", [4, P, 2])
    wed = dt("wed", [4, P, 1]);  ebd = dt("ebd", [1, 1])
    eb1 = dt("eb1", [P, 8]); eg1 = dt("eg1", [P, 8]); et1 = dt("et1", [P, 8])
    eb2 = dt("eb2", [P, 8]); eg2 = dt("eg2", [P, 8]); et2 = dt("et2", [P, 8])
    eb3 = dt("eb3", [P, 4]); eg3 = dt("eg3", [P, 4]); et3 = dt("et3", [P, 4])
    eb4 = dt("eb4", [1, 2])
    eo_d = dt("eo", [C, 2], "ExternalOutput")

    with tile.TileContext(nc) as tc:
        with tc.tile_pool(name="pool", bufs=1) as pool, \
             tc.tile_pool(name="psum", bufs=2, space="PSUM") as psum, \
             tc.tile_pool(name="const", bufs=1) as const:
            ones_row = const.tile([1, P], F32)
            nc.vector.memset(ones_row, 1.0)
            inv1024 = const.tile([P, P], F32)
            nc.vector.memset(inv1024, 1.0 / 1024.0)
            inv512 = const.tile([P, P], F32)
            nc.vector.memset(inv512, 1.0 / 512.0)
            eps_s = const.tile([P, 1], F32)
            nc.vector.memset(eps_s, EPS)
            xg_s = pool.tile([P, D // P, C], F32R, name="xg_s", tag="xg")
            nc.sync.dma_start(xg_s, xg)
            small = {}
            for nm, d_ap, sh in [("eb1", eb1, 8), ("eg1", eg1, 8),
                                 ("et1", et1, 8), ("eb2", eb2, 8),
                                 ("eg2", eg2, 8), ("et2", et2, 8),
                                 ("eb3", eb3, 4), ("eg3", eg3, 4),
                                 ("et3", et3, 4)]:
                t = pool.tile([P, sh], F32, name=nm + "s", tag=nm + "s")
                nc.sync.dma_start(t, d_ap)
                small[nm] = t
            eb4_s = pool.tile([1, 2], F32, name="eb4s", tag="eb4s")
            nc.sync.dma_start(eb4_s, eb4)
            we4_s = pool.tile([P, 4, 2], F32, name="we4s", tag="we4s")
            nc.sync.dma_start(we4_s, we4.rearrange("k p e -> p k e"))
            # we2 resident, loaded early to overlap with L1 compute
            we2_s = pool.tile([P, 8, 1024], F32R, name="we2_s", tag="we2")
            nc.sync.dma_start(we2_s, we2)

            x1 = _ln_mlp_layer(nc, pool, psum, xg_s, we1, small["eb1"],
                               small["eg1"], small["et1"], D // P, 8, C,
                               inv1024, eps_s, "e1", wgroups=2)
            x2 = _ln_mlp_layer(nc, pool, psum, x1, we2_s, small["eb2"],
                               small["eg2"], small["et2"], 8, 8, C,
                               inv1024, eps_s, "e2", wgroups=0)
            x3 = _ln_mlp_layer(nc, pool, psum, x2, we3, small["eb3"],
                               small["eg3"], small["et3"], 8, 4, C,
                               inv512, eps_s, "e3", wgroups=2, out_round=False)
            _head_logsoftmax(nc, pool, psum, x3,
                             [we4_s[:, k, :] for k in range(4)],
                             eb4_s, eo_d, 4, 2, C, ones_row, "eo")
    nc.compile()
    return nc


_BUILT = {}
_PREP = {}
_LAST_EXEC_NS = None  # sum of per-phase max-core exec times, when traced


def _fingerprint(arrs):
    h = 0
    for a in arrs:
        b = np.ascontiguousarray(a).view(np.uint8)
        h = hash((h, a.shape, bytes(b[:: max(1, b.size // 64)][:64].tobytes())))
    return h


def _run(nc, maps):
    """run_bass_kernel_spmd with graceful fallback when the axon NTFF
    profiling hook is unavailable but BASS_TRACE is set."""
    import os
    try:
        return run_bass_kernel_spmd(nc, maps, core_ids=list(range(NCORES)))
    except ModuleNotFoundError:
        os.environ["BASS_NEVER_TRACE"] = "1"
        return run_bass_kernel_spmd(nc, maps, core_ids=list(range(NCORES)))


def _get(key, fn):
    if key not in _BUILT:
        _BUILT[key] = fn()
    return _BUILT[key]


# ---------------------------------------------------------------- host prep

def _prep_w(w):
    """[K, F] -> [128(=ki part), K//128, F] contiguous."""
    K, F = w.shape
    return np.ascontiguousarray(w.reshape(K // P, P, F).transpose(1, 0, 2))


def _prep_b(b):
    F = b.shape[0]
    return np.ascontiguousarray(b.reshape(F // P, P).T)


def _prep_xt(x):
    """[N, D] -> [128, D//128, N] feature-major."""
    N = x.shape[0]
    return np.ascontiguousarray(
        x.T.reshape(D // P, P, N).transpose(1, 0, 2))


def _fold_bn(w, b, g, beta, m, v):
    a = (g / np.sqrt(v + EPS)).astype(np.float32)
    return (w * a[None, :]).astype(np.float32), \
        ((b - m) * a + beta).astype(np.float32)


def kernel(**inputs):
    f = {k: np.asarray(v, dtype=np.float32) for k, v in inputs.items()}
    source = f["source"]

    # ---- phase A inputs
    w1f, b1f = _fold_bn(f["bw1"], f["bb1"], f["bg1"], f["bbe1"], f["bm1"], f["bv1"])
    w2f, b2f = _fold_bn(f["bw2"], f["bb2"], f["bg2"], f["bbe2"], f["bm2"], f["bv2"])
    d1f, db1f = _fold_bn(f["dw1"], f["db1"], f["dg1"], f["dbe1"], f["dm1"], f["dv1"])
    d2f, db2f = _fold_bn(f["dw2"], f["db2"], f["dg2"], f["dbe2"], f["dm2"], f["dv2"])
    d3f, db3f = _fold_bn(f["dw3"], f["db3"], f["dg3"], f["dbe3"], f["dm3"], f["dv3"])
    shared = {
        "w1": _prep_w(w1f), "b1": _prep_b(b1f),
        "w2": _prep_w(w2f), "b2": _prep_b(b2f),
        "cw": np.ascontiguousarray(f["cw"].reshape(2, P, E)),
        "cb": f["cb"].reshape(1, E).copy(),
        "dw1": _prep_w(d1f).astype(ml_dtypes.bfloat16),
        "db1": _prep_b(db1f),
        "dw2": _prep_w(d2f).astype(ml_dtypes.bfloat16),
        "db2": _prep_b(db2f),
        "dw3": _prep_w(d3f).astype(ml_dtypes.bfloat16),
        "db3": _prep_b(db3f),
        "dw4": np.ascontiguousarray(f["dw4"].reshape(4, P, 2)),
        "db4": f["db4"].reshape(1, 2).copy(),
    }
    nc_a = _get("a", _build_phase_a)
    in_maps_a = []
    for c in range(NCORES):
        m = dict(shared)
        m["xt"] = _prep_xt(source[c * BSH:(c + 1) * BSH])
        in_maps_a.append(m)
    res_a = _run(nc_a, in_maps_a)
    src = np.concatenate([r["src"] for r in res_a.results], axis=0)
    dclf = np.concatenate([r["dclf"] for r in res_a.results], axis=0)

    # ---- routing on host (argmax of device-computed log-softmax)
    label = np.argmax(src, axis=-1)
    idx = [np.nonzero(label == e)[0] for e in range(E)]
    maxn = max(1, max(len(i) for i in idx))
    C = max(256, ((maxn + 63) // 64) * 64)

    nc_b = _get(("b", C), lambda: _build_phase_b(C))
    in_maps_b = []
    for e in range(E):
        g = np.zeros((C, D), dtype=np.float32)
        g[:len(idx[e])] = source[idx[e]]
        in_maps_b.append({
            "xg": _prep_xt(g),
            "we1": _prep_w(f["ew1"][e]), "we2": _prep_w(f["ew2"][e]),
            "we3": _prep_w(f["ew3"][e]),
            "we4": np.ascontiguousarray(f["ew4"][e].reshape(4, P, 2)),
            "eb1": _prep_b(f["eb1"][e]), "eg1": _prep_b(f["eg1"][e]),
            "et1": _prep_b(f["ebe1"][e]),
            "eb2": _prep_b(f["eb2"][e]), "eg2": _prep_b(f["eg2"][e]),
            "et2": _prep_b(f["ebe2"][e]),
            "eb3": _prep_b(f["eb3"][e]), "eg3": _prep_b(f["eg3"][e]),
            "et3": _prep_b(f["ebe3"][e]),
            "eb4": f["eb4"][e].reshape(1, 2).copy(),
        })
    res_b = _run(nc_b, in_maps_b)
    sout = np.zeros((B, 2), dtype=np.float32)
    for e in range(E):
        if len(idx[e]):
            sout[idx[e]] = res_b.results[e]["eo"][:len(idx[e])]

    global _LAST_EXEC_NS
    if res_a.exec_time_ns is not None and res_b.exec_time_ns is not None:
        _LAST_EXEC_NS = res_a.exec_time_ns + res_b.exec_time_ns
    return src, dclf, sout, source
